# revision 1
# baseline (speedup 1.0000x reference)
"""GATv2 (3-layer, 4-head, GraphNorm) Bass kernel for 8 trn2 NeuronCores.

Sharding: nodes partitioned by dst across 8 cores. Each core computes the
full xl projection table (replicated), gathers xl[src]/xr[dst] per
128-dst-node block via SWDGE dma_gather, does block-batched edge math,
segment softmax + aggregation via selection-matrix matmuls in PSUM, then
GraphNorm with an AllReduce for global stats and an AllGather of
transposed node-feature shards feeding the next layer's projections.
"""
import math

import ml_dtypes
import numpy as np

import concourse.bacc as bacc
import concourse.bass as bass
import concourse.tile as tile
from concourse import mybir
from concourse.bass_utils import run_bass_kernel_spmd
from concourse.masks import make_identity

F32 = mybir.dt.float32
BF16 = mybir.dt.bfloat16
I16 = mybir.dt.int16
I32 = mybir.dt.int32
AF = mybir.ActivationFunctionType
ALU = mybir.AluOpType

NC = 8
D = 64
H = 4
C = 64
HC = H * C  # 256
L = 3
NEG = 0.2
EPS = 1e-5
P = 128


def _bf(x):
    return np.asarray(x, dtype=ml_dtypes.bfloat16)


def _wrap_idx(idx):
    """[n*128] int -> [128, n*8] int16 wrapped in 16 partitions, replicated
    across the 8 gpsimd core groups."""
    n = idx.shape[0]
    assert n % 128 == 0
    w = idx.reshape(n // 16, 16).T  # [16, n//16]
    return np.tile(w, (8, 1)).astype(np.int16)


def preprocess(inputs):
    """Host-side: shard/sort/pad edges, build all per-core input tensors."""
    x = np.asarray(inputs["x"], np.float32)
    ei = np.asarray(inputs["edge_index"], np.int64)
    Wl = np.asarray(inputs["Wl"], np.float32)
    bl = np.asarray(inputs["bl"], np.float32)
    Wr = np.asarray(inputs["Wr"], np.float32)
    br = np.asarray(inputs["br"], np.float32)
    att = np.asarray(inputs["att"], np.float32)
    conv_bias = np.asarray(inputs["conv_bias"], np.float32)
    gn_weight = np.asarray(inputs["gn_weight"], np.float32)
    gn_scale = np.asarray(inputs["gn_scale"], np.float32)
    gn_bias = np.asarray(inputs["gn_bias"], np.float32)

    N = x.shape[0]
    NSH = N // NC
    NBLK = (NSH + P - 1) // P
    RW = NBLK * P

    loop = np.arange(N, dtype=np.int64)
    src = np.concatenate([ei[0], loop])
    dst = np.concatenate([ei[1], loop])

    per_core = []
    cnts = np.zeros((NC, NBLK), np.int64)
    for c in range(NC):
        sel = (dst >= c * NSH) & (dst < (c + 1) * NSH)
        s = src[sel].astype(np.int32)
        dl = (dst[sel] - c * NSH).astype(np.int32)
        order = np.argsort(dl, kind="stable")
        s, dl = s[order], dl[order]
        blk = dl // P
        starts = np.searchsorted(blk, np.arange(NBLK))
        ends = np.searchsorted(blk, np.arange(NBLK), side="right")
        cnts[c] = ends - starts
        per_core.append((s, dl, starts, ends))

    nchunk = [max(1, int(math.ceil(cnts[:, b].max() / P))) for b in range(NBLK)]
    IWC = int(sum(nchunk))
    cum = np.concatenate([[0], np.cumsum(nchunk)]).astype(int)

    in_maps = []
    for c in range(NC):
        s, dl, starts, ends = per_core[c]
        srcw = np.zeros((P, IWC * 8), np.int16)
        dlocr = np.full(IWC * P, 255.0, np.float32)
        dloc = np.full((P, IWC), 255.0, np.float32)
        for b in range(NBLK):
            ns = nchunk[b] * P
            e0, e1 = starts[b], ends[b]
            sp = np.zeros(ns, np.int16)
            lp = np.full(ns, 255.0, np.float32)
            n = e1 - e0
            sp[:n] = s[e0:e1]
            lp[:n] = (dl[e0:e1] - b * P).astype(np.float32)
            co = int(cum[b]) * 8
            srcw[:, co : co + nchunk[b] * 8] = _wrap_idx(sp)
            dlocr[int(cum[b]) * P : int(cum[b]) * P + ns] = lp
            dloc[:, int(cum[b]) : int(cum[b + 1])] = lp.reshape(nchunk[b], P).T
        in_maps.append(
            {"srcw": srcw, "dlocr": _bf(dlocr), "dloc": _bf(dloc)}
        )

    wts = np.zeros((L, 2, D + 1, HC), np.float32)
    for l in range(L):
        wts[l, 0, :D] = Wl[l].T
        wts[l, 0, D] = bl[l]
        wts[l, 1, :D] = Wr[l].T
        wts[l, 1, D] = br[l]
    wts = _bf(wts)

    attb = _bf(att.reshape(L, HC))

    gnc = np.stack(
        [
            conv_bias,
            2 * conv_bias,
            conv_bias * conv_bias,
            gn_scale * (2 - gn_scale),
            gn_scale,
            gn_weight,
            gn_bias,
        ],
        axis=1,
    ).astype(np.float32)  # [L, 7, C]

    xt0 = np.zeros((P, 4 * RW), np.float32)
    for r in range(NC):
        hr, ir = r // 4, r % 4
        xt0[hr * D : hr * D + D, ir * RW : ir * RW + NSH] = x[
            r * NSH : (r + 1) * NSH
        ].T
    xt0 = _bf(xt0)

    for c in range(NC):
        in_maps[c]["wts"] = wts
        in_maps[c]["attb"] = attb
        in_maps[c]["gnc"] = gnc
        in_maps[c]["xt0"] = xt0
        xtme = np.zeros((D, RW), np.float32)
        xtme[:, :NSH] = x[c * NSH : (c + 1) * NSH].T
        in_maps[c]["xtme0"] = _bf(xtme)

    cfg = dict(
        N=N, NSH=NSH, NBLK=NBLK, RW=RW, nchunk=[int(v) for v in nchunk],
        cum=[int(v) for v in cum], IWC=IWC,
    )
    return cfg, in_maps


def _ap3(ap, d1, d2):
    """Build [P, d1, d2] AP from a 2D AP by appending explicit dims."""
    return bass.AP(tensor=ap.tensor, offset=ap.offset, ap=[list(ap.ap[0]), d1, d2])


def build(cfg):
    N, NSH, NBLK, RW = cfg["N"], cfg["NSH"], cfg["NBLK"], cfg["RW"]
    nchunk, cum, IWC = cfg["nchunk"], cfg["cum"], cfg["IWC"]
    NT = N + (-N) % P
    nRT = (NSH + P - 1) // P

    nc = bacc.Bacc("TRN2", target_bir_lowering=False, debug=False, num_devices=NC)

    srcw = nc.dram_tensor("srcw", [P, IWC * 8], I16, kind="ExternalInput").ap()
    dlocr = nc.dram_tensor("dlocr", [IWC * P], BF16, kind="ExternalInput").ap()
    dloc = nc.dram_tensor("dloc", [P, IWC], BF16, kind="ExternalInput").ap()
    wts = nc.dram_tensor("wts", [L, 2, D + 1, HC], BF16, kind="ExternalInput").ap()
    attb = nc.dram_tensor("attb", [L, HC], BF16, kind="ExternalInput").ap()
    gnc = nc.dram_tensor("gnc", [L, 7, C], F32, kind="ExternalInput").ap()
    xt0 = nc.dram_tensor("xt0", [P, 4 * RW], BF16, kind="ExternalInput").ap()
    xtme0 = nc.dram_tensor("xtme0", [D, RW], BF16, kind="ExternalInput").ap()
    out = nc.dram_tensor("out", [NSH, C], F32, kind="ExternalOutput").ap()

    xl_tab = nc.dram_tensor("xl_tab", [NT, HC], BF16).ap()
    arin = [nc.dram_tensor(f"arin{l}", [P], F32).ap() for l in range(L)]
    arout = [
        nc.dram_tensor(f"arout{l}", [P], F32, addr_space="Shared").ap()
        for l in range(L)
    ]
    agin = [nc.dram_tensor(f"agin{l}", [D, NSH], F32).ap() for l in range(L - 1)]
    agout = [
        nc.dram_tensor(f"agout{l}", [NC, D, NSH], F32, addr_space="Shared").ap()
        for l in range(L - 1)
    ]

    with tile.TileContext(nc) as tc:
        with (
            tc.tile_pool(name="res", bufs=1) as res,
            tc.tile_pool(name="big", bufs=2) as big,
            tc.tile_pool(name="med", bufs=3) as med,
            tc.tile_pool(name="sm", bufs=2) as sm,
            tc.tile_pool(name="ps", bufs=2, space="PSUM") as ps,
            tc.tile_pool(name="psa", bufs=2, space="PSUM") as psa,
            tc.tile_pool(name="psb", bufs=1, space="PSUM") as psb,
            tc.tile_pool(name="psx", bufs=2, space="PSUM") as psx,
        ):
            # ---- resident loads / constants ----
            src_sb = res.tile([P, IWC * 8], I16)
            nc.sync.dma_start(out=src_sb[:], in_=srcw[:, :])
            dloc_sb = res.tile([P, IWC], BF16)
            nc.sync.dma_start(out=dloc_sb[:], in_=dloc[:, :])

            iota_i = res.tile([P, P], I32)
            nc.gpsimd.iota(iota_i[:], pattern=[[1, P]], base=0, channel_multiplier=0)
            iota_row = res.tile([P, P], BF16)
            nc.vector.tensor_copy(out=iota_row[:], in_=iota_i[:])

            iota_ci = res.tile([P, P], I32)
            nc.gpsimd.iota(iota_ci[:], pattern=[[0, P]], base=0,
                           channel_multiplier=1)
            iota_rep = res.tile([P, P], BF16)
            nc.vector.tensor_copy(out=iota_rep[:], in_=iota_ci[:])
            xr_res = res.tile([P, NBLK, HC], BF16)
            ones_row = res.tile([1, P], BF16)
            nc.vector.memset(ones_row[:], 1.0)
            ones_col = res.tile([P, 1], F32)
            nc.vector.memset(ones_col[:], 1.0)
            ident = res.tile([P, P], F32)
            make_identity(nc, ident[:])
            eps_col = res.tile([P, 1], F32)
            nc.vector.memset(eps_col[:], EPS)

            xt_pack = res.tile([P, 4 * RW], BF16)
            nc.sync.dma_start(out=xt_pack[:], in_=xt0[:, :])
            xtme_sb = res.tile([D, RW], BF16)
            nc.sync.dma_start(out=xtme_sb[:], in_=xtme0[:, :])

            w_tiles = {}
            b_tiles = {}
            for l in range(L):
                for side in range(2):
                    # weights duplicated into both partition halves so lhsT
                    # slices based at partition 0 or 64 both find a matching
                    # rhs base
                    t = res.tile([P, HC], BF16, tag=f"w{l}{side}")
                    nc.sync.dma_start(out=t[:D, :], in_=wts[l, side, :D, :])
                    nc.sync.dma_start(out=t[D:, :], in_=wts[l, side, :D, :])
                    w_tiles[(l, side)] = t
                    bt = res.tile([1, HC], BF16, tag=f"b{l}{side}")
                    nc.sync.dma_start(out=bt[:], in_=wts[l, side, D : D + 1, :])
                    b_tiles[(l, side)] = bt

            att_bc = {}
            for l in range(L):
                t = res.tile([P, HC], BF16, tag=f"att{l}")
                nc.sync.dma_start(
                    out=t[:],
                    in_=bass.AP(
                        tensor=attb.tensor, offset=attb.offset + l * HC,
                        ap=[[0, P], [1, HC]],
                    ),
                )
                att_bc[l] = t

            gnc_bc = {}
            for l in range(L):
                t = res.tile([P, 7, C], F32, tag=f"gnc{l}")
                nc.sync.dma_start(
                    out=t[:],
                    in_=bass.AP(
                        tensor=gnc.tensor, offset=gnc.offset + l * 7 * C,
                        ap=[[0, P], [C, 7], [1, C]],
                    ),
                )
                gnc_bc[l] = t

            h_big = res.tile([P, NBLK, C], F32)
            xtsh_sb = res.tile([D, RW], F32)
            nc.vector.memset(xtsh_sb[:], 0.0)

            for l in range(L):
                # ================= projections =================
                for r in range(NC):
                    hr, ir = r // 4, r % 4
                    for j in range(nRT):
                        n0 = j * P
                        cnt = min(P, NSH - n0)
                        lhsT = xt_pack[
                            hr * D : hr * D + D, ir * RW + n0 : ir * RW + n0 + P
                        ]
                        pt = ps.tile([P, HC], F32, tag="pj", space="PSUM")
                        nc.tensor.matmul(
                            out=pt[:], lhsT=lhsT,
                            rhs=w_tiles[(l, 0)][hr * D : hr * D + D, :],
                            start=True, stop=False,
                        )
                        nc.tensor.matmul(
                            out=pt[:], lhsT=ones_row[:],
                            rhs=b_tiles[(l, 0)][:],
                            start=False, stop=True,
                        )
                        st = med.tile([P, HC], BF16, tag="pjsb")
                        nc.scalar.activation(st[:], pt[:], AF.Copy)
                        g0 = r * NSH + n0
                        nc.sync.dma_start(
                            out=xl_tab[g0 : g0 + cnt, :], in_=st[:cnt, :]
                        )
                for j in range(nRT):
                    n0 = j * P
                    lhsT = xtme_sb[:, n0 : n0 + P]
                    pt = ps.tile([P, HC], F32, tag="pj", space="PSUM")
                    nc.tensor.matmul(
                        out=pt[:], lhsT=lhsT, rhs=w_tiles[(l, 1)][:D, :],
                        start=True, stop=False,
                    )
                    nc.tensor.matmul(
                        out=pt[:], lhsT=ones_row[:],
                        rhs=b_tiles[(l, 1)][:],
                        start=False, stop=True,
                    )
                    nc.scalar.activation(xr_res[:, j, :], pt[:], AF.Copy)

                # ================= edge blocks =================
                stats_ps = psb.tile([P, 1], F32, tag="stats", space="PSUM")
                for b in range(NBLK):
                    nch = nchunk[b]
                    nidx = nch * P
                    co = cum[b]

                    # dma_gather tops out at 1024 indices (64 descriptors
                    # per SDMA engine) -- split into sub-calls
                    xl_g = big.tile([P, nch, HC], BF16, tag="xlg")
                    for k in range(0, nch, 8):
                        kn = min(8, nch - k)
                        sub = kn * P
                        nc.gpsimd.dma_gather(
                            out_ap=xl_g[:, k : k + kn, :], in_ap=xl_tab[:, :],
                            idxs_ap=src_sb[
                                :, (co + k) * 8 : (co + k + kn) * 8
                            ],
                            num_idxs=sub, num_idxs_reg=sub, elem_size=HC,
                        )
                    # partition-replicated local-dst row (edge-major)
                    drow = sm.tile([P, nch * P], BF16, tag="drow")
                    nc.sync.dma_start(
                        out=drow[:],
                        in_=bass.AP(
                            tensor=dlocr.tensor, offset=dlocr.offset + co * P,
                            ap=[[0, P], [1, nch * P]],
                        ),
                    )
                    # S[i, e] = (i == dloc[e]) : [128, nch, 128]
                    s_all = sm.tile([P, nch, P], BF16, tag="sall")
                    nc.vector.tensor_tensor(
                        out=s_all[:],
                        in0=_ap3(iota_rep[:], [0, nch], [1, P]),
                        in1=_ap3(drow[:], [P, nch], [1, P]),
                        op=ALU.is_equal,
                    )
                    # xr[dst] per edge via PE: xr_exp = S.T @ xr_blk
                    xr_g = big.tile([P, nch, HC], BF16, tag="g1")
                    for j in range(nch):
                        xre = psx.tile([P, HC], F32, tag="xre", space="PSUM")
                        nc.tensor.matmul(
                            out=xre[:], lhsT=s_all[:, j, :],
                            rhs=xr_res[:, b, :],
                            start=True, stop=True,
                        )
                        nc.scalar.activation(xr_g[:, j, :], xre[:], AF.Copy)

                    st_all = sm.tile([P, nch, P], BF16, tag="st")
                    dl_ap = dloc_sb[:, co : co + nch]
                    nc.vector.tensor_tensor(
                        out=st_all[:],
                        in0=_ap3(dl_ap, list(dl_ap.ap[1]), [0, P]),
                        in1=_ap3(iota_row[:], [0, nch], [1, P]),
                        op=ALU.is_equal,
                    )

                    u = big.tile([P, nch, HC], BF16, tag="g2")
                    nc.vector.tensor_add(out=u[:], in0=xl_g[:], in1=xr_g[:])
                    lr = big.tile([P, nch, HC], BF16, tag="g1")
                    nc.scalar.activation(lr[:], u[:], AF.Prelu, alpha=NEG)
                    v = big.tile([P, nch, HC], BF16, tag="g2")
                    ab = att_bc[l][:]
                    nc.vector.tensor_mul(
                        out=v[:], in0=lr[:], in1=_ap3(ab, [0, nch], [1, HC])
                    )
                    logits = sm.tile([P, nch, H], F32, tag="lg")
                    vv = v[:].rearrange("p n (h c) -> p n h c", h=H)
                    for h in range(H):
                        nc.vector.tensor_reduce(
                            out=logits[:, :, h],
                            in_=vv[:, :, h, :],
                            axis=mybir.AxisListType.X,
                            op=ALU.add,
                        )
                    # wcat: cols 0:H hold a=exp(logits), cols H: hold a*xl
                    wcat = big.tile([P, nch, H + HC], BF16, tag="g2")
                    nc.scalar.activation(wcat[:, :, :H], logits[:], AF.Exp)
                    nc.vector.tensor_mul(
                        out=wcat[:, :, H:].rearrange("p n (h c) -> p n h c", h=H),
                        in0=xl_g[:].rearrange("p n (h c) -> p n h c", h=H),
                        in1=wcat[:, :, :H].to_broadcast([P, nch, H, C]),
                    )

                    agg_ps = psa.tile([P, H + HC], F32, tag="agg", space="PSUM")
                    for j in range(nch):
                        nc.tensor.matmul(
                            out=agg_ps[:], lhsT=st_all[:, j, :], rhs=wcat[:, j, :],
                            start=(j == 0), stop=(j == nch - 1),
                        )

                    # epilogue: h_blk = mean_h(agg/den) (conv_bias folded
                    # into the GraphNorm affine)
                    den4 = sm.tile([P, H], F32, tag="d4")
                    nc.scalar.activation(
                        den4[:], agg_ps[:, :H], AF.Copy, scale=float(H),
                        bias=1e-12,
                    )
                    rec4 = sm.tile([P, H], F32, tag="rc")
                    nc.vector.reciprocal(out=rec4[:], in_=den4[:])
                    sc = sm.tile([P, HC], F32, tag="sc")
                    nc.vector.tensor_mul(
                        out=sc[:].rearrange("p (h c) -> p h c", h=H),
                        in0=agg_ps[:, H:].rearrange("p (h c) -> p h c", h=H),
                        in1=rec4[:].to_broadcast([P, H, C]),
                    )
                    nc.vector.tensor_reduce(
                        out=h_big[:, b, :],
                        in_=_ap3(sc[:], [1, C], [C, H]),
                        axis=mybir.AxisListType.X,
                        op=ALU.add,
                    )
                    hcat = sm.tile([P, 2 * C], F32, tag="hcat")
                    nc.vector.tensor_copy(out=hcat[:, :C], in_=h_big[:, b, :])
                    nc.vector.tensor_mul(
                        out=hcat[:, C:], in0=h_big[:, b, :], in1=h_big[:, b, :]
                    )
                    nc.tensor.matmul(
                        out=stats_ps[:], lhsT=hcat[:], rhs=ones_col[:],
                        start=(b == 0), stop=(b == NBLK - 1),
                    )

                # ================= GraphNorm =================
                stats_sb = sm.tile([P, 1], F32, tag="stsb")
                nc.scalar.activation(stats_sb[:], stats_ps[:], AF.Copy)
                nc.sync.dma_start(out=arin[l][:, None], in_=stats_sb[:])
                nc.gpsimd.collective_compute(
                    "AllReduce", ALU.add,
                    ins=[arin[l].opt()], outs=[arout[l].opt()],
                    replica_groups=[list(range(NC))],
                )
                srow = sm.tile([P, P], F32, tag="srow")
                nc.sync.dma_start(
                    out=srow[:],
                    in_=bass.AP(
                        tensor=arout[l].tensor, offset=arout[l].offset,
                        ap=[[0, P], [1, P]],
                    ),
                )
                g = gnc_bc[l]
                invN = 1.0 / float(N)
                m1 = sm.tile([P, C], F32, tag="m1")
                nc.scalar.activation(m1[:], srow[:, 0:C], AF.Copy, scale=invN)
                m2 = sm.tile([P, C], F32, tag="m2")
                nc.scalar.activation(m2[:], srow[:, C : 2 * C], AF.Copy, scale=invN)
                mu = sm.tile([P, C], F32, tag="mu")
                nc.vector.tensor_add(out=mu[:], in0=m1[:], in1=g[:, 0, :])
                t1 = sm.tile([P, C], F32, tag="t1")
                nc.vector.tensor_mul(out=t1[:], in0=mu[:], in1=mu[:])
                t2 = sm.tile([P, C], F32, tag="t2")
                nc.vector.tensor_mul(out=t2[:], in0=t1[:], in1=g[:, 3, :])
                u1 = sm.tile([P, C], F32, tag="u1")
                nc.vector.tensor_mul(out=u1[:], in0=m1[:], in1=g[:, 1, :])
                eh2 = sm.tile([P, C], F32, tag="eh2")
                nc.vector.tensor_add(out=eh2[:], in0=m2[:], in1=u1[:])
                nc.vector.tensor_add(out=eh2[:], in0=eh2[:], in1=g[:, 2, :])
                var = sm.tile([P, C], F32, tag="var")
                nc.vector.tensor_tensor(
                    out=var[:], in0=eh2[:], in1=t2[:], op=ALU.subtract
                )
                srt = sm.tile([P, C], F32, tag="srt")
                nc.scalar.activation(srt[:], var[:], AF.Sqrt, bias=eps_col[:])
                rst = sm.tile([P, C], F32, tag="rst")
                nc.vector.reciprocal(out=rst[:], in_=srt[:])
                A = sm.tile([P, C], F32, tag="A")
                nc.vector.tensor_mul(out=A[:], in0=rst[:], in1=g[:, 5, :])
                q = sm.tile([P, C], F32, tag="q")
                nc.vector.tensor_mul(out=q[:], in0=mu[:], in1=g[:, 4, :])
                nc.vector.tensor_tensor(
                    out=q[:], in0=g[:, 0, :], in1=q[:], op=ALU.subtract
                )
                Bt = sm.tile([P, C], F32, tag="B")
                nc.vector.tensor_mul(out=Bt[:], in0=A[:], in1=q[:])
                nc.vector.tensor_add(out=Bt[:], in0=Bt[:], in1=g[:, 6, :])

                for b in range(NBLK):
                    cnt = min(P, NSH - b * P)
                    xb = sm.tile([P, C], F32, tag="xb")
                    nc.vector.tensor_mul(out=xb[:], in0=h_big[:, b, :], in1=A[:])
                    nc.vector.tensor_add(out=xb[:], in0=xb[:], in1=Bt[:])
                    if l == L - 1:
                        nc.sync.dma_start(
                            out=out[b * P : b * P + cnt, :], in_=xb[:cnt, :]
                        )
                    else:
                        tp = ps.tile([D, P], F32, tag="pj", space="PSUM")
                        nc.tensor.transpose(out=tp[:], in_=xb[:], identity=ident[:])
                        nc.vector.tensor_copy(
                            out=xtsh_sb[:, b * P : b * P + P], in_=tp[:]
                        )
                if l < L - 1:
                    nc.sync.dma_start(out=agin[l][:, :], in_=xtsh_sb[:, :NSH])
                    nc.gpsimd.collective_compute(
                        "AllGather", ALU.bypass,
                        ins=[agin[l].opt()], outs=[agout[l].opt()],
                        replica_groups=[list(range(NC))],
                    )
                    for r in range(NC):
                        hr, ir = r // 4, r % 4
                        nc.gpsimd.dma_start(
                            out=xt_pack[
                                hr * D : hr * D + D, ir * RW : ir * RW + NSH
                            ],
                            in_=agout[l][r, :, :],
                        )
                    nc.vector.tensor_copy(out=xtme_sb[:], in_=xtsh_sb[:])

    nc.compile()
    return nc


_CACHE = {}


def kernel(**inputs):
    cfg, in_maps = preprocess(inputs)
    key = (cfg["N"], tuple(cfg["nchunk"]))
    if key not in _CACHE:
        _CACHE[key] = build(cfg)
    nc = _CACHE[key]
    res = run_bass_kernel_spmd(nc, in_maps, core_ids=list(range(NC)))
    shards = [res.results[c]["out"] for c in range(NC)]
    return np.concatenate(shards, axis=0).astype(np.float32)


def _install_ntff_hook():
    import sys, types
    try:
        from antenv.axon_hooks import get_axon_ntff_profile_hook  # noqa
        return
    except ImportError:
        pass
    import trn_agent_boot.trn_boot as tb
    mod = types.ModuleType("antenv.axon_hooks")
    _hook = [None]
    mod.set_axon_ntff_profile_hook = lambda h: _hook.__setitem__(0, h)
    mod.get_axon_ntff_profile_hook = lambda: _hook[0]
    sys.modules["antenv.axon_hooks"] = mod
    import antenv
    antenv.axon_hooks = mod
    mod.set_axon_ntff_profile_hook(
        tb._ntff_profile_via_ctypes("/opt/axon/libaxon_pjrt.so")
    )


def run_traced(**inputs):
    """Re-run the cached kernel with NTFF tracing; returns exec_time_ns."""
    _install_ntff_hook()
    cfg, in_maps = preprocess(inputs)
    key = (cfg["N"], tuple(cfg["nchunk"]))
    if key not in _CACHE:
        _CACHE[key] = build(cfg)
    nc = _CACHE[key]
    res = run_bass_kernel_spmd(
        nc, in_maps, core_ids=list(range(NC)), trace=True
    )
    return res.exec_time_ns

